# revision 7
# baseline (speedup 1.0000x reference)
"""CondConv (per-sample routed 3x3 conv) on 8 Trainium2 NeuronCores.

Reference computation (all fp32):
    gap     = mean(x, axis=(2,3))                    [B, CIN]
    routing = sigmoid(gap @ W_att.T + b_att)         [B, E]
    ker     = einsum('be,eoihw->boihw', routing, convs)
    out[b]  = conv2d(x[b], ker[b], stride 1, pad 1)  [B, COUT, 56, 56]

Sharding (B=32, COUT=256 across 8 cores): 4 core-pairs; pair p owns
samples 8p..8p+7 (batch data-parallel), and within a pair each core
computes one half of COUT (128 channels). Halving COUT per core halves
the resident expert bank so the whole pipeline stays fp32 in SBUF.

Per-core program (SPMD — same program, different data):
  - expert bank convsT [8e][2c][128cin, 9*128] resident in SBUF
  - per sample: DMA padded x -> GAP via ScalarE accum -> routing
    (2 small matmuls + sigmoid + diag/broadcast matmul) -> VectorE mixes
    the per-sample kernel with fused scalar_tensor_tensor -> conv as
    2c*9shift*7tile accumulating fp32r matmuls (N=448) -> ScalarE drains
    PSUM -> DMA out.
"""

import numpy as np

B, CIN, H, W = 32, 256, 56, 56
COUT, KK, E = 256, 3, 8
HP, WP = H + 2, W + 2          # zero-padded input plane
PHW = HP * WP                  # 3364
NSH = KK * KK                  # 9 shifts
CHUNKS = 2                     # CIN = 2 * 128
MHALF = COUT // 2              # couts per core
ROWS_PER_TILE = 8              # output rows per matmul tile
NTILES = H // ROWS_PER_TILE    # 7
NFREE = ROWS_PER_TILE * W      # 448
NCORES = 8
SAMPLES_PER_CORE = B // (NCORES // 2)  # 8

_cached = {}


def _build_program():
    import concourse.bacc as bacc
    import concourse.mybir as mybir
    from concourse.tile import TileContext

    f32 = mybir.dt.float32
    f32r = mybir.dt.float32r
    Alu = mybir.AluOpType
    Act = mybir.ActivationFunctionType

    nc = bacc.Bacc(None, target_bir_lowering=False)

    xpad_d = nc.declare_dram_parameter(
        "xpad", [SAMPLES_PER_CORE, CHUNKS, 128, PHW], f32, isOutput=False)
    convsT_d = nc.declare_dram_parameter(
        "convsT", [E, CHUNKS, 128, NSH * 128], f32, isOutput=False)
    watt_d = nc.declare_dram_parameter("watt", [CHUNKS, 128, E], f32, isOutput=False)
    batt_d = nc.declare_dram_parameter("batt", [E, 1], f32, isOutput=False)
    ones8_d = nc.declare_dram_parameter("ones8", [E, 128], f32, isOutput=False)
    ident8_d = nc.declare_dram_parameter("ident8", [E, E], f32, isOutput=False)
    out_d = nc.declare_dram_parameter(
        "out", [SAMPLES_PER_CORE, MHALF, H, W], f32, isOutput=True)

    with TileContext(nc) as tc:
        with (
            tc.tile_pool(name="resident", bufs=1) as res_pool,
            tc.tile_pool(name="xp", bufs=3) as xp_pool,
            tc.tile_pool(name="kt", bufs=3) as kt_pool,
            tc.tile_pool(name="small", bufs=3) as small_pool,
            tc.tile_pool(name="outsb", bufs=4) as out_pool,
            tc.tile_pool(name="cpsum", bufs=1, space="PSUM") as cps_pool,
            tc.tile_pool(name="rpsum", bufs=1, space="PSUM") as rps_pool,
        ):
            # ---- resident tiles -------------------------------------------------
            convsT_sb = [[None] * CHUNKS for _ in range(E)]
            for e in range(E):
                for c in range(CHUNKS):
                    t = res_pool.tile([128, NSH * 128], f32,
                                      name=f"cv_{e}_{c}", tag=f"cv_{e}_{c}")
                    nc.sync.dma_start(out=t[:], in_=convsT_d[e, c])
                    convsT_sb[e][c] = t
            watt_sb = []
            for c in range(CHUNKS):
                t = res_pool.tile([128, E], f32, name=f"watt{c}", tag=f"watt{c}")
                nc.sync.dma_start(out=t[:], in_=watt_d[c])
                watt_sb.append(t)
            batt_sb = res_pool.tile([E, 1], f32, name="batt", tag="batt")
            nc.sync.dma_start(out=batt_sb[:], in_=batt_d[:])
            ones8_sb = res_pool.tile([E, 128], f32, name="ones8", tag="ones8")
            nc.sync.dma_start(out=ones8_sb[:], in_=ones8_d[:])
            ident8_sb = res_pool.tile([E, E], f32, name="ident8", tag="ident8")
            nc.sync.dma_start(out=ident8_sb[:], in_=ident8_d[:])
            # broadcast routing weights: scal[:, 8*b+e] = r_be on every partition
            scal_sb = res_pool.tile([128, SAMPLES_PER_CORE * E], f32,
                                    name="scal", tag="scal")

            for b in range(SAMPLES_PER_CORE):
                # ---- load padded input ----------------------------------------
                # ScalarE in-place Copy rounds fp32 -> fp32r for the conv
                # matmuls (walrus requires fp32r matmul inputs to come from a
                # rounding producer) and its accum_out yields the GAP row sums
                # in the same pass.
                xp = []
                gs = []
                for c in range(CHUNKS):
                    t = xp_pool.tile([128, PHW], f32r, name=f"xp{c}", tag=f"xp{c}")
                    nc.gpsimd.dma_start(out=t[:], in_=xpad_d[b, c])
                    g = small_pool.tile([128, 1], f32, name=f"gs{c}", tag=f"gs{c}")
                    nc.scalar.activation(out=t[:], in_=t[:], func=Act.Copy,
                                         accum_out=g[:])
                    xp.append(t)
                    gs.append(g)

                # ---- routing ---------------------------------------------------
                # logits[e] = sum_cin gap*W/3136 + b  (1/3136 folded into watt)
                ps_r = rps_pool.tile([E, 1], f32, name="ps_r", tag="rps")
                for c in range(CHUNKS):
                    nc.tensor.matmul(ps_r[:], watt_sb[c][:], gs[c][:],
                                     start=(c == 0), stop=(c == CHUNKS - 1))
                rout = small_pool.tile([E, 1], f32, name="rout", tag="rout")
                nc.scalar.activation(out=rout[:], in_=ps_r[:], func=Act.Sigmoid,
                                     bias=batt_sb[:, 0:1], scale=1.0)
                # diag(r) then ones.T @ diag(r) broadcasts r to all 128 partitions
                diag = small_pool.tile([E, E], f32, name="diag", tag="diag")
                nc.vector.tensor_scalar_mul(out=diag[:], in0=ident8_sb[:],
                                            scalar1=rout[:, 0:1])
                ps_b = rps_pool.tile([128, E], f32, name="ps_b", tag="rps")
                nc.tensor.matmul(ps_b[:], ones8_sb[:], diag[:], start=True, stop=True)
                nc.scalar.activation(out=scal_sb[:, b * E:(b + 1) * E], in_=ps_b[:],
                                     func=Act.Copy)

                # ---- mix per-sample kernel on VectorE --------------------------
                # kerT[c][cin, s*128+m] = sum_e r_be * convsT[e][c][cin, s*128+m]
                kt = []
                for c in range(CHUNKS):
                    k = kt_pool.tile([128, NSH * 128], f32r, name=f"kt{c}", tag=f"kt{c}")
                    nc.vector.tensor_scalar_mul(
                        out=k[:], in0=convsT_sb[0][c][:],
                        scalar1=scal_sb[:, b * E:b * E + 1])
                    for e in range(1, E):
                        nc.vector.scalar_tensor_tensor(
                            out=k[:], in0=convsT_sb[e][c][:],
                            scalar=scal_sb[:, b * E + e:b * E + e + 1],
                            in1=k[:], op0=Alu.mult, op1=Alu.add)
                    kt.append(k)

                # ---- conv: accumulate 2c * 9shift into 7 PSUM tiles ------------
                cps = [cps_pool.tile([128, NFREE], f32, name=f"cps{n}", tag=f"cps{n}")
                       for n in range(NTILES)]
                for c in range(CHUNKS):
                    x3 = xp[c].rearrange("p (r q) -> p r q", q=WP)
                    for s in range(NSH):
                        dh, dw = s // KK, s % KK
                        lhsT = kt[c][:, s * 128:(s + 1) * 128]
                        first = (c == 0 and s == 0)
                        last = (c == CHUNKS - 1 and s == NSH - 1)
                        for n in range(NTILES):
                            rhs = x3[:, n * ROWS_PER_TILE + dh:
                                     n * ROWS_PER_TILE + dh + ROWS_PER_TILE,
                                     dw:dw + W]
                            nc.tensor.matmul(cps[n][:], lhsT, rhs,
                                             start=first, stop=last)

                # ---- drain + store --------------------------------------------
                for n in range(NTILES):
                    o = out_pool.tile([128, NFREE], f32, name="osb", tag="osb")
                    nc.scalar.activation(out=o[:], in_=cps[n][:], func=Act.Copy)
                    nc.sync.dma_start(
                        out=out_d[b, :, n * ROWS_PER_TILE:(n + 1) * ROWS_PER_TILE, :],
                        in_=o[:])

    nc.compile()
    return nc


def _prep_core_inputs(x, convs, W_att, b_att):
    """Host-side shard/layout prep. Returns list of 8 per-core input dicts."""
    f32 = np.float32
    # padded input, cin split into 2 chunks of 128
    xpad = np.zeros((B, CHUNKS, 128, HP, WP), dtype=f32)
    xpad[:, :, :, 1:H + 1, 1:W + 1] = np.ascontiguousarray(x, dtype=f32).reshape(
        B, CHUNKS, 128, H, W)
    xpad = xpad.reshape(B, CHUNKS, 128, PHW)

    # convsT[half][e, c, cin, s*128 + m] = convs[e, half*128+m, c*128+cin, kh, kw]
    cv = np.ascontiguousarray(convs, dtype=f32).reshape(E, 2, MHALF, CHUNKS, 128, NSH)
    convsT_halves = [
        np.ascontiguousarray(cv[:, h].transpose(0, 2, 3, 4, 1).reshape(
            E, CHUNKS, 128, NSH * 128))
        for h in range(2)
    ]

    watt = np.ascontiguousarray(
        (np.asarray(W_att, dtype=f32).T / f32(H * W)).reshape(CHUNKS, 128, E))
    batt = np.ascontiguousarray(np.asarray(b_att, dtype=f32).reshape(E, 1))
    ones8 = np.ones((E, 128), dtype=f32)
    ident8 = np.eye(E, dtype=f32)

    in_maps = []
    for k in range(NCORES):
        pair, half = k // 2, k % 2
        sl = slice(pair * SAMPLES_PER_CORE, (pair + 1) * SAMPLES_PER_CORE)
        in_maps.append({
            "xpad": np.ascontiguousarray(xpad[sl]),
            "convsT": convsT_halves[half],
            "watt": watt,
            "batt": batt,
            "ones8": ones8,
            "ident8": ident8,
        })
    return in_maps


def _assemble_output(results):
    out = np.empty((B, COUT, H, W), dtype=np.float32)
    for k in range(NCORES):
        pair, half = k // 2, k % 2
        sl = slice(pair * SAMPLES_PER_CORE, (pair + 1) * SAMPLES_PER_CORE)
        out[sl, half * MHALF:(half + 1) * MHALF] = results[k]["out"]
    return out


def kernel(x, convs, W_att, b_att):
    from concourse.bass_utils import run_bass_kernel_spmd

    if "nc" not in _cached:
        _cached["nc"] = _build_program()
    in_maps = _prep_core_inputs(x, convs, W_att, b_att)
    res = run_bass_kernel_spmd(_cached["nc"], in_maps, core_ids=list(range(NCORES)))
    return _assemble_output(res.results)


# revision 8
# speedup vs baseline: 1.3104x; 1.3104x over previous
"""CondConv (per-sample routed 3x3 conv) on 8 Trainium2 NeuronCores.

Reference computation (all fp32):
    gap     = mean(x, axis=(2,3))                    [B, CIN]
    routing = sigmoid(gap @ W_att.T + b_att)         [B, E]
    ker     = einsum('be,eoihw->boihw', routing, convs)
    out[b]  = conv2d(x[b], ker[b], stride 1, pad 1)  [B, COUT, 56, 56]

Sharding (B=32, COUT=256 across 8 cores): 4 core-pairs; pair p owns
samples 8p..8p+7 (batch data-parallel), and within a pair each core
computes one half of COUT (128 channels). Halving COUT per core halves
the resident expert bank so the whole pipeline stays fp32 in SBUF.

Per-core program (SPMD — same program, different data):
  - expert bank convsT [8e][2c][128cin, 9*128] resident in SBUF
  - per sample: DMA padded x -> GAP via ScalarE accum -> routing
    (2 small matmuls + sigmoid + diag/broadcast matmul) -> VectorE mixes
    the per-sample kernel with fused scalar_tensor_tensor -> conv as
    2c*9shift*7tile accumulating fp32r matmuls (N=448) -> ScalarE drains
    PSUM -> DMA out.
"""

import numpy as np

B, CIN, H, W = 32, 256, 56, 56
COUT, KK, E = 256, 3, 8
HP, WP = H + 2, W + 2          # zero-padded input plane
PHW = HP * WP                  # 3364
NSH = KK * KK                  # 9 shifts
CHUNKS = 2                     # CIN = 2 * 128
MHALF = COUT // 2              # couts per core
ROWS_PER_TILE = 8              # output rows per matmul tile
NTILES = H // ROWS_PER_TILE    # 7
NFREE = ROWS_PER_TILE * W      # 448
NCORES = 8
SAMPLES_PER_CORE = B // (NCORES // 2)  # 8

_cached = {}


def _build_program():
    import concourse.bacc as bacc
    import concourse.mybir as mybir
    from concourse.tile import TileContext

    f32 = mybir.dt.float32
    f32r = mybir.dt.float32r
    Alu = mybir.AluOpType
    Act = mybir.ActivationFunctionType

    nc = bacc.Bacc(None, target_bir_lowering=False)

    xpad_d = nc.declare_dram_parameter(
        "xpad", [SAMPLES_PER_CORE, CHUNKS, 128, PHW], f32, isOutput=False)
    convsT_d = nc.declare_dram_parameter(
        "convsT", [E, CHUNKS, 128, NSH * 128], f32, isOutput=False)
    watt_d = nc.declare_dram_parameter("watt", [CHUNKS, 128, E], f32, isOutput=False)
    batt_d = nc.declare_dram_parameter("batt", [E, 1], f32, isOutput=False)
    ones8_d = nc.declare_dram_parameter("ones8", [E, 128], f32, isOutput=False)
    ident8_d = nc.declare_dram_parameter("ident8", [E, E], f32, isOutput=False)
    out_d = nc.declare_dram_parameter(
        "out", [SAMPLES_PER_CORE, MHALF, H, W], f32, isOutput=True)

    with TileContext(nc) as tc:
        with (
            tc.tile_pool(name="resident", bufs=1) as res_pool,
            tc.tile_pool(name="xp", bufs=3) as xp_pool,
            tc.tile_pool(name="kt", bufs=3) as kt_pool,
            tc.tile_pool(name="small", bufs=3) as small_pool,
            tc.tile_pool(name="outsb", bufs=4) as out_pool,
            tc.tile_pool(name="cpsum", bufs=1, space="PSUM") as cps_pool,
            tc.tile_pool(name="rpsum", bufs=1, space="PSUM") as rps_pool,
        ):
            # ---- resident tiles -------------------------------------------------
            convsT_sb = [[None] * CHUNKS for _ in range(E)]
            for e in range(E):
                for c in range(CHUNKS):
                    t = res_pool.tile([128, NSH * 128], f32,
                                      name=f"cv_{e}_{c}", tag=f"cv_{e}_{c}")
                    nc.sync.dma_start(out=t[:], in_=convsT_d[e, c])
                    convsT_sb[e][c] = t
            watt_sb = []
            for c in range(CHUNKS):
                t = res_pool.tile([128, E], f32, name=f"watt{c}", tag=f"watt{c}")
                nc.sync.dma_start(out=t[:], in_=watt_d[c])
                watt_sb.append(t)
            batt_sb = res_pool.tile([E, 1], f32, name="batt", tag="batt")
            nc.sync.dma_start(out=batt_sb[:], in_=batt_d[:])
            ones8_sb = res_pool.tile([E, 128], f32, name="ones8", tag="ones8")
            nc.sync.dma_start(out=ones8_sb[:], in_=ones8_d[:])
            ident8_sb = res_pool.tile([E, E], f32, name="ident8", tag="ident8")
            nc.sync.dma_start(out=ident8_sb[:], in_=ident8_d[:])
            # broadcast routing weights: scal[:, 8*b+e] = r_be on every partition
            scal_sb = res_pool.tile([128, SAMPLES_PER_CORE * E], f32,
                                    name="scal", tag="scal")

            def emit_route_mix(b):
                """Load + GAP + routing + per-sample kernel mix for sample b."""
                # ScalarE in-place Copy rounds fp32 -> fp32r for the conv
                # matmuls (walrus requires fp32r matmul inputs to come from a
                # rounding producer) and its accum_out yields the GAP row sums
                # in the same pass.
                xp = []
                gs = []
                for c in range(CHUNKS):
                    t = xp_pool.tile([128, PHW], f32r, name=f"xp{c}", tag=f"xp{c}")
                    nc.gpsimd.dma_start(out=t[:], in_=xpad_d[b, c])
                    g = small_pool.tile([128, 1], f32, name=f"gs{c}", tag=f"gs{c}")
                    nc.scalar.activation(out=t[:], in_=t[:], func=Act.Copy,
                                         accum_out=g[:])
                    xp.append(t)
                    gs.append(g)

                # routing: logits[e] = sum_cin gap*W/3136 + b (1/3136 in watt)
                ps_r = rps_pool.tile([E, 1], f32, name="ps_r", tag="rps")
                for c in range(CHUNKS):
                    nc.tensor.matmul(ps_r[:], watt_sb[c][:], gs[c][:],
                                     start=(c == 0), stop=(c == CHUNKS - 1))
                rout = small_pool.tile([E, 1], f32, name="rout", tag="rout")
                nc.scalar.activation(out=rout[:], in_=ps_r[:], func=Act.Sigmoid,
                                     bias=batt_sb[:, 0:1], scale=1.0)
                # diag(r) then ones.T @ diag(r) broadcasts r to all partitions
                diag = small_pool.tile([E, E], f32, name="diag", tag="diag")
                nc.vector.tensor_scalar_mul(out=diag[:], in0=ident8_sb[:],
                                            scalar1=rout[:, 0:1])
                ps_b = rps_pool.tile([128, E], f32, name="ps_b", tag="rps")
                nc.tensor.matmul(ps_b[:], ones8_sb[:], diag[:], start=True, stop=True)
                nc.scalar.activation(out=scal_sb[:, b * E:(b + 1) * E], in_=ps_b[:],
                                     func=Act.Copy)

                # mix on VectorE:
                # kerT[c][cin, s*128+m] = sum_e r_be * convsT[e][c][cin, s*128+m]
                kt = []
                for c in range(CHUNKS):
                    k = kt_pool.tile([128, NSH * 128], f32r, name=f"kt{c}", tag=f"kt{c}")
                    nc.vector.tensor_scalar_mul(
                        out=k[:], in0=convsT_sb[0][c][:],
                        scalar1=scal_sb[:, b * E:b * E + 1])
                    for e in range(1, E):
                        nc.vector.scalar_tensor_tensor(
                            out=k[:], in0=convsT_sb[e][c][:],
                            scalar=scal_sb[:, b * E + e:b * E + e + 1],
                            in1=k[:], op0=Alu.mult, op1=Alu.add)
                    kt.append(k)
                return xp, kt

            def emit_conv(b, xp, kt):
                """Conv for sample b: accumulate 2c*9shift into 7 PSUM tiles."""
                cps = [cps_pool.tile([128, NFREE], f32, name=f"cps{n}", tag=f"cps{n}")
                       for n in range(NTILES)]
                for c in range(CHUNKS):
                    x3 = xp[c].rearrange("p (r q) -> p r q", q=WP)
                    for s in range(NSH):
                        dh, dw = s // KK, s % KK
                        lhsT = kt[c][:, s * 128:(s + 1) * 128]
                        first = (c == 0 and s == 0)
                        last = (c == CHUNKS - 1 and s == NSH - 1)
                        for n in range(NTILES):
                            rhs = x3[:, n * ROWS_PER_TILE + dh:
                                     n * ROWS_PER_TILE + dh + ROWS_PER_TILE,
                                     dw:dw + W]
                            nc.tensor.matmul(cps[n][:], lhsT, rhs,
                                             start=first, stop=last)
                for n in range(NTILES):
                    o = out_pool.tile([128, NFREE], f32, name="osb", tag="osb")
                    nc.scalar.activation(out=o[:], in_=cps[n][:], func=Act.Copy)
                    nc.sync.dma_start(
                        out=out_d[b, :, n * ROWS_PER_TILE:(n + 1) * ROWS_PER_TILE, :],
                        in_=o[:])

            # Software-pipelined emission: sample b's routing + mix is issued
            # LOOKAHEAD samples before conv(b), so the tiny routing matmuls of
            # upcoming samples sit ahead of the long conv bursts in the PE
            # queue and the VectorE mix overlaps earlier convs instead of
            # serializing after them.
            LOOKAHEAD = 3
            stash = {b: emit_route_mix(b) for b in range(min(LOOKAHEAD, SAMPLES_PER_CORE))}
            for b in range(SAMPLES_PER_CORE):
                emit_conv(b, *stash.pop(b))
                if b + LOOKAHEAD < SAMPLES_PER_CORE:
                    stash[b + LOOKAHEAD] = emit_route_mix(b + LOOKAHEAD)

    nc.compile()
    return nc


def _prep_core_inputs(x, convs, W_att, b_att):
    """Host-side shard/layout prep. Returns list of 8 per-core input dicts."""
    f32 = np.float32
    # padded input, cin split into 2 chunks of 128
    xpad = np.zeros((B, CHUNKS, 128, HP, WP), dtype=f32)
    xpad[:, :, :, 1:H + 1, 1:W + 1] = np.ascontiguousarray(x, dtype=f32).reshape(
        B, CHUNKS, 128, H, W)
    xpad = xpad.reshape(B, CHUNKS, 128, PHW)

    # convsT[half][e, c, cin, s*128 + m] = convs[e, half*128+m, c*128+cin, kh, kw]
    cv = np.ascontiguousarray(convs, dtype=f32).reshape(E, 2, MHALF, CHUNKS, 128, NSH)
    convsT_halves = [
        np.ascontiguousarray(cv[:, h].transpose(0, 2, 3, 4, 1).reshape(
            E, CHUNKS, 128, NSH * 128))
        for h in range(2)
    ]

    watt = np.ascontiguousarray(
        (np.asarray(W_att, dtype=f32).T / f32(H * W)).reshape(CHUNKS, 128, E))
    batt = np.ascontiguousarray(np.asarray(b_att, dtype=f32).reshape(E, 1))
    ones8 = np.ones((E, 128), dtype=f32)
    ident8 = np.eye(E, dtype=f32)

    in_maps = []
    for k in range(NCORES):
        pair, half = k // 2, k % 2
        sl = slice(pair * SAMPLES_PER_CORE, (pair + 1) * SAMPLES_PER_CORE)
        in_maps.append({
            "xpad": np.ascontiguousarray(xpad[sl]),
            "convsT": convsT_halves[half],
            "watt": watt,
            "batt": batt,
            "ones8": ones8,
            "ident8": ident8,
        })
    return in_maps


def _assemble_output(results):
    out = np.empty((B, COUT, H, W), dtype=np.float32)
    for k in range(NCORES):
        pair, half = k // 2, k % 2
        sl = slice(pair * SAMPLES_PER_CORE, (pair + 1) * SAMPLES_PER_CORE)
        out[sl, half * MHALF:(half + 1) * MHALF] = results[k]["out"]
    return out


def kernel(x, convs, W_att, b_att):
    from concourse.bass_utils import run_bass_kernel_spmd

    if "nc" not in _cached:
        _cached["nc"] = _build_program()
    in_maps = _prep_core_inputs(x, convs, W_att, b_att)
    res = run_bass_kernel_spmd(_cached["nc"], in_maps, core_ids=list(range(NCORES)))
    return _assemble_output(res.results)
